# revision 8
# baseline (speedup 1.0000x reference)
"""GCNConv + PReLU on Trainium2, 8-core SPMD Bass/Tile kernel.

Math (PyG GCNConv, add_self_loops=True, symmetric norm), matching the
reference:
    h = x @ W
    deg[c] = (# edges with col == c) + 1          (self-loop)
    dis = rsqrt(deg)
    out[c] = dis[c] * ( sum_{e: col_e == c} dis[row_e] * h[row_e]
                        + dis[c] * h[c] )         (self-loop term)
             + bias
    z = prelu(out)

Distribution (hardcoded, per the sharding hint): destination nodes are
sharded across the 8 cores (12500 nodes each); W/bias/prelu are
replicated; every core computes the full g = dis*h table locally
(x replicated, bf16) so the per-edge bulk gather of source features is
local.  Edges are bucketed/sorted by destination on the host (index-only
work) and packed into 128-edge tiles targeting 128-node destination
windows.  Because the bulk-gather primitive (dma_gather) takes int16
indices, the g table is split into 4 sub-tables of 2 shards
(25088 rows < 2^15) and each window's tiles are grouped by source
sub-table; tiles are laid out sub-major so each sub-table is gathered in
big contiguous chunks.

Destination degrees are integer metadata of the host edge bucketing and
are shipped as counts; all float math (rsqrt, x@W, scaling, PReLU) runs
on device.

Device pipeline per core:
  A) dis = 1/sqrt(deg_counts + 1) for all shards (one activation+recip).
  B) g-table build: h = x @ W tile-by-tile in bf16 (x host-transposed so
     x tiles feed the PE as lhsT), row-scaled by dis on the Activation
     engine, written to a DRAM table.
  C) bulk dma_gather of source rows (16-tile chunks per sub-table);
     per tile a one-hot(edge -> dest-in-window) built on DVE (is_equal
     vs iota, all-bf16) and a PE matmul onehot^T @ gathered accumulating
     the window's [128 dest, 128 feat] sums in PSUM; per window: add
     self-loop g, scale by dis (Activation engine), add bias, PReLU via
     max(y, a*y), DMA to the out shard.
"""

import math
import sys

for _p in ("/opt/trn_rl_repo",):
    if _p not in sys.path:
        sys.path.insert(0, _p)

import numpy as np
import ml_dtypes

BF16 = ml_dtypes.bfloat16

P = 128
CORES = 8
NSUB = 4  # g-table split into 4 sub-tables (2 shards each) for int16 idx
GCT = 32  # tiles per dma_gather chunk
MCH = 512  # colrel tiles per sbuf chunk
IDXCH = 1024  # idx16 columns per sbuf chunk (= 128 tiles)
XCH = 512  # x columns per load

FULL_CFG = dict(N=100000, F_IN=256, F_OUT=128, E=1600000)

_prog_cache = {}


def _derived(cfg):
    N = cfg["N"]
    shard = N // CORES
    nw = math.ceil(shard / P)
    gstride = nw * P  # padded rows per shard in the g table
    return shard, nw, gstride


def _schedule(tsw):
    """tsw[w][s] = tiles for (window w, sub-table s), identical on all
    cores. Tiles are laid out sub-major: all of sub 0's tiles (in window
    order), then sub 1's, ... with each sub-stream padded to a multiple
    of GCT tiles (dummy tiles are gathered but never consumed)."""
    nw = len(tsw)
    tsub = [sum(tsw[w][s] for w in range(nw)) for s in range(NSUB)]
    tsub_pad = [-(-x // GCT) * GCT for x in tsub]
    S = [0] * (NSUB + 1)
    for s in range(NSUB):
        S[s + 1] = S[s] + tsub_pad[s]
    pre = [[0] * (nw + 1) for _ in range(NSUB)]
    for s in range(NSUB):
        for w in range(nw):
            pre[s][w + 1] = pre[s][w] + tsw[w][s]
    return S, pre, S[NSUB]  # sub starts, per-sub window presums, T


def host_prep(x, edge_index, W, bias, prelu_a, cfg):
    """Pure index/layout prep: shard + sort edges by destination (and by
    source sub-table within each destination window), pad into fixed
    128-edge tiles, build the int16 wrapped gather-index array, count
    per-destination edges (integer bucketing metadata), transpose x and
    cast to bf16. No float math on input values beyond the dtype cast."""
    N, F_IN, F_OUT = cfg["N"], cfg["F_IN"], cfg["F_OUT"]
    shard, nw, gstride = _derived(cfg)

    x = np.asarray(x, np.float32)
    W = np.asarray(W, np.float32)
    bias = np.asarray(bias, np.float32)
    prelu_a = np.asarray(prelu_a, np.float32)
    ei = np.asarray(edge_index)

    order = np.argsort(ei[1], kind="stable")
    rs = np.asarray(ei[0])[order].astype(np.int64)
    cs = np.asarray(ei[1])[order].astype(np.int64)
    bounds = np.searchsorted(cs, np.arange(CORES + 1) * shard)

    # integer destination-degree counts (edge bucketing metadata), laid
    # out [P, CORES*nw] so column k*nw+w row p = count of node
    # k*shard + w*P + p; shard-tail pad rows are 0.
    deg = np.bincount(cs, minlength=N).astype(np.float32)
    dl = np.zeros((CORES, nw * P), np.float32)
    for k in range(CORES):
        dl[k, :shard] = deg[k * shard : (k + 1) * shard]
    deg_lay = np.ascontiguousarray(
        dl.reshape(CORES, nw, P).transpose(2, 0, 1).reshape(P, CORES * nw)
    )

    shards_per_sub = max(1, CORES // NSUB)
    cnts = np.zeros((CORES, nw, NSUB), np.int64)
    per_core = []
    for k in range(CORES):
        seg = slice(bounds[k], bounds[k + 1])
        local = cs[seg] - k * shard
        w_arr = local // P
        s_arr = rs[seg] // (shards_per_sub * shard)
        key = w_arr * NSUB + s_arr
        o2 = np.argsort(key, kind="stable")
        cnts[k] = np.bincount(key, minlength=nw * NSUB).reshape(nw, NSUB)
        per_core.append((local[o2], w_arr[o2], s_arr[o2], rs[seg][o2], key[o2]))

    tsw = (-(-cnts // P)).max(axis=0)  # [nw, NSUB]
    tsw[:, 0] = np.maximum(tsw[:, 0], 1)  # every window needs >= 1 matmul
    tsw_l = [[int(tsw[w][s]) for s in range(NSUB)] for w in range(nw)]
    S, pre, T = _schedule(tsw_l)

    tile_base = np.zeros((nw, NSUB), np.int64)
    for w in range(nw):
        for s in range(NSUB):
            tile_base[w, s] = S[s] + pre[s][w]

    colrel = np.full((CORES, T * P), -1.0, np.float32)
    idx16 = np.zeros((CORES, 16, T * 8), np.int16)
    for k in range(CORES):
        local, w_arr, s_arr, r_arr, key = per_core[k]
        cnt_flat = cnts[k].reshape(-1)
        gstart = np.concatenate([[0], np.cumsum(cnt_flat)])
        within = np.arange(local.size) - gstart[key]
        slot = tile_base[w_arr, s_arr] * P + within
        colrel[k, slot] = (local - w_arr * P).astype(np.float32)
        rk = r_arr // shard
        v = ((rk % shards_per_sub) * gstride + (r_arr - rk * shard)).astype(np.int16)
        idx16[k, slot % 16, (slot // P) * 8 + (slot % P) // 16] = v

    colrel_t16 = np.ascontiguousarray(
        colrel.reshape(CORES, T, P).transpose(0, 2, 1).astype(BF16)
    )
    idx16_rep = np.ascontiguousarray(np.tile(idx16, (1, P // 16, 1)))  # [CORES,128,T*8]

    return dict(
        tsw=tuple(tuple(r) for r in tsw_l),
        T=T,
        x_t=np.ascontiguousarray(x.T.astype(BF16)),
        w=W.astype(BF16),
        bias_b=np.ascontiguousarray(np.tile(bias[None, :], (P, 1))),
        prelu_b=np.ascontiguousarray(np.tile(prelu_a[None, :], (P, 1))),
        deg_lay=deg_lay,
        idx16=idx16_rep,
        colrel_t16=colrel_t16,
    )


def build_program(cfg, tsw, debug_outs=False):
    """Build + compile the SPMD Bass program (same for all 8 cores)."""
    import concourse.bass as bass
    import concourse.bacc as bacc
    import concourse.mybir as mybir
    import concourse.tile as tile
    from concourse.bass import ds

    f32 = mybir.dt.float32
    bf16 = mybir.dt.bfloat16
    i16 = mybir.dt.int16
    AOT = mybir.AluOpType
    ACT = mybir.ActivationFunctionType

    N, F_IN, F_OUT = cfg["N"], cfg["F_IN"], cfg["F_OUT"]
    shard, nw, gstride = _derived(cfg)
    kchunks = F_IN // P
    shards_per_sub = max(1, CORES // NSUB)
    S, pre, T = _schedule(tsw)

    nc = bacc.Bacc(
        "TRN2",
        target_bir_lowering=False,
        debug=False,
        num_devices=CORES,
        num_swdge_queues=4,
    )

    x_t = nc.dram_tensor("x_t", [F_IN, N], bf16, kind="ExternalInput")
    w_d = nc.dram_tensor("w", [F_IN, F_OUT], bf16, kind="ExternalInput")
    bias_d = nc.dram_tensor("bias_b", [P, F_OUT], f32, kind="ExternalInput")
    prelu_d = nc.dram_tensor("prelu_b", [P, F_OUT], f32, kind="ExternalInput")
    deg_d = nc.dram_tensor("deg_lay", [P, CORES * nw], f32, kind="ExternalInput")
    idx16_d = nc.dram_tensor("idx16", [P, T * 8], i16, kind="ExternalInput")
    colrel16_d = nc.dram_tensor("colrel_t16", [P, T], bf16, kind="ExternalInput")
    out_d = nc.dram_tensor("out", [shard, F_OUT], f32, kind="ExternalOutput")

    g_subs = [
        nc.dram_tensor(f"g_sub{s}", [shards_per_sub * gstride, F_OUT], bf16)
        for s in range(NSUB)
    ]

    dbg_agg_d = None
    if debug_outs:
        dbg_agg_d = nc.dram_tensor(
            "dbg_agg", [nw * P, F_OUT], f32, kind="ExternalOutput"
        )

    with tile.TileContext(nc) as tc:
        with (
            tc.tile_pool(name="const", bufs=1) as constp,
            tc.tile_pool(name="deg", bufs=1) as degp,
            tc.tile_pool(name="gown", bufs=1) as gownp,
            tc.tile_pool(name="accw", bufs=1) as accwp,
            tc.tile_pool(name="b_x", bufs=4) as bxp,
            tc.tile_pool(name="b_ps", bufs=2, space="PSUM") as bpsp,
            tc.tile_pool(name="b_g", bufs=4) as bgp,
            tc.tile_pool(name="c_col", bufs=2) as ccolp,
            tc.tile_pool(name="c_idx", bufs=2) as cidxp,
            tc.tile_pool(name="c_g", bufs=3) as cgp,
            tc.tile_pool(name="c_oh", bufs=8) as cohp,
            tc.tile_pool(name="c_ps", bufs=4, space="PSUM") as cpsp,
            tc.tile_pool(name="c_f", bufs=4) as cfp,
        ):
            iota4 = constp.tile([P, 4 * P], bf16)
            nc.gpsimd.iota(
                iota4[:],
                pattern=[[0, 4], [1, P]],
                base=0,
                channel_multiplier=0,
                allow_small_or_imprecise_dtypes=True,
            )
            wt = []
            for c in range(kchunks):
                wc = constp.tile([P, F_OUT], bf16, tag=f"wc{c}")
                nc.sync.dma_start(out=wc[:], in_=w_d[c * P : (c + 1) * P, :])
                wt.append(wc)
            biasb = constp.tile([P, F_OUT], f32)
            nc.sync.dma_start(out=biasb[:], in_=bias_d[:, :])
            prelub = constp.tile([P, F_OUT], f32)
            nc.sync.dma_start(out=prelub[:], in_=prelu_d[:, :])

            # ---------------- dis = 1/sqrt(deg+1), all shards -----------
            pid = nc.partition_id()
            dis_all = degp.tile([P, CORES * nw], f32)
            nc.sync.dma_start(out=dis_all[:], in_=deg_d[:, :])
            nc.scalar.activation(
                out=dis_all[:], in_=dis_all[:], func=ACT.Sqrt, bias=1.0, scale=1.0
            )
            nc.vector.reciprocal(out=dis_all[:], in_=dis_all[:])
            dis_s = degp.tile([P, nw], f32)
            nc.sync.dma_start(out=dis_s[:], in_=deg_d[:, ds(pid * nw, nw)])
            nc.scalar.activation(
                out=dis_s[:], in_=dis_s[:], func=ACT.Sqrt, bias=1.0, scale=1.0
            )
            nc.vector.reciprocal(out=dis_s[:], in_=dis_s[:])

            # zero the per-shard padding rows of the sub-tables (never
            # gathered by real indices, but keep the memory finite)
            if gstride > shard:
                with tc.tile_pool(name="b_z", bufs=1) as bzp:
                    zt = bzp.tile([P, F_OUT], bf16)
                    nc.vector.memset(zt[:], 0.0)
                    for s in range(NSUB):
                        for b in range(shards_per_sub):
                            nc.sync.dma_start(
                                out=g_subs[s][
                                    b * gstride + shard : (b + 1) * gstride, :
                                ],
                                in_=zt[: gstride - shard, :],
                            )

            gown = gownp.tile([P, nw * F_OUT], f32)
            accw = accwp.tile([P, nw * F_OUT], f32)
            if nw * P > shard:
                # tail-window partitions beyond the shard are read (and
                # discarded) by the flush path; keep them finite
                nc.vector.memset(gown[:, (nw - 1) * F_OUT : nw * F_OUT], 0.0)

            def phase_b_shard(k):
                """g table rows for destination shard k = dis * (x @ W)."""
                with nc.named_scope(f"phaseB{k}"):
                    for c0 in range(0, shard, XCH):
                        cl = min(XCH, shard - c0)
                        xts = []
                        for c in range(kchunks):
                            xt = bxp.tile([P, XCH], bf16, tag=f"xt{c}")
                            nc.sync.dma_start(
                                out=xt[:, :cl],
                                in_=x_t[
                                    c * P : (c + 1) * P,
                                    k * shard + c0 : k * shard + c0 + cl,
                                ],
                            )
                            xts.append(xt)
                        gt = bgp.tile([P, 4 * F_OUT], bf16, tag="bg")
                        nfull = 0
                        for s0 in range(0, cl, P):
                            nn = min(P, cl - s0)
                            nt = (c0 + s0) // P
                            j = s0 // P
                            ph = bpsp.tile([P, F_OUT], f32, tag="bps")
                            for c in range(kchunks):
                                nc.tensor.matmul(
                                    out=ph[:nn, :],
                                    lhsT=xts[c][:, s0 : s0 + nn],
                                    rhs=wt[c][:],
                                    start=(c == 0),
                                    stop=(c == kchunks - 1),
                                )
                            nc.scalar.activation(
                                out=gt[:nn, j * F_OUT : (j + 1) * F_OUT],
                                in_=ph[:nn, :],
                                func=ACT.Copy,
                                scale=dis_all[:nn, k * nw + nt : k * nw + nt + 1],
                            )
                            if nn == P:
                                nfull = j + 1
                        rb = (k % shards_per_sub) * gstride + c0
                        sub_t = g_subs[k // shards_per_sub]
                        if nfull:
                            nc.sync.dma_start(
                                out=sub_t[rb : rb + nfull * P, :].rearrange(
                                    "(j p) f -> p j f", p=P
                                ),
                                in_=gt[:, : nfull * F_OUT].rearrange(
                                    "p (j f) -> p j f", f=F_OUT
                                ),
                            )
                        if cl > nfull * P:
                            nn = cl - nfull * P
                            nc.sync.dma_start(
                                out=sub_t[rb + nfull * P : rb + cl, :],
                                in_=gt[:nn, nfull * F_OUT : (nfull + 1) * F_OUT],
                            )

            def phase_own():
                """own-shard g recompute (self-loop term), pid-dynamic x."""
                with (
                    nc.named_scope("phaseOwn"),
                    tc.tile_pool(name="o_x", bufs=4) as oxp,
                    tc.tile_pool(name="o_ps", bufs=2, space="PSUM") as opsp,
                ):
                    for c0 in range(0, shard, XCH):
                        cl = min(XCH, shard - c0)
                        xts = []
                        for c in range(kchunks):
                            xt = oxp.tile([P, XCH], bf16, tag=f"oxt{c}")
                            nc.sync.dma_start(
                                out=xt[:, :cl],
                                in_=x_t[c * P : (c + 1) * P, ds(pid * shard + c0, cl)],
                            )
                            xts.append(xt)
                        for s0 in range(0, cl, P):
                            nn = min(P, cl - s0)
                            w = (c0 + s0) // P
                            ph = opsp.tile([P, F_OUT], f32, tag="ops")
                            for c in range(kchunks):
                                nc.tensor.matmul(
                                    out=ph[:nn, :],
                                    lhsT=xts[c][:, s0 : s0 + nn],
                                    rhs=wt[c][:],
                                    start=(c == 0),
                                    stop=(c == kchunks - 1),
                                )
                            nc.scalar.activation(
                                out=gown[:nn, w * F_OUT : (w + 1) * F_OUT],
                                in_=ph[:nn, :],
                                func=ACT.Copy,
                                scale=dis_s[:nn, w : w + 1],
                            )

            # ---------------- Phase C: gather + scatter matmuls ---------
            gq = [0]

            def phase_c_sub(s):
                """process all windows' tiles of sub-table s; accumulate
                into accw (s=0 init with gown; s=NSUB-1 flush)."""
                ccol, ccol_rng = None, (-1, -1)
                cidx, cidx_rng = None, (-1, -1)
                gch, gch_rng = None, (-1, -1)
                with nc.named_scope(f"phaseC{s}"):
                    for w in range(nw):
                        nt = tsw[w][s]
                        pw = None
                        if nt:
                            pw = cpsp.tile([P, F_OUT], f32, tag="cps")
                        for j0 in range(0, nt, 4):
                            gl = min(4, nt - j0)
                            t0 = S[s] + pre[s][w] + j0
                            if not (ccol_rng[0] <= t0 + gl - 1 < ccol_rng[1]):
                                c0 = (t0 // MCH) * MCH
                                cl = min(MCH, T - c0)
                                ccol = ccolp.tile([P, MCH], bf16, tag="cc", name="cc")
                                nc.sync.dma_start(
                                    out=ccol[:, :cl],
                                    in_=colrel16_d[:, c0 : c0 + cl],
                                )
                                ccol_rng = (c0, c0 + cl)
                            oh4 = cohp.tile([P, 4 * P], bf16, tag="coh")
                            i0 = t0 - ccol_rng[0]
                            nc.vector.tensor_tensor(
                                out=oh4[:, : gl * P].rearrange(
                                    "p (g q) -> p g q", g=gl
                                ),
                                in0=iota4[:, : gl * P].rearrange(
                                    "p (g q) -> p g q", g=gl
                                ),
                                in1=ccol[:, i0 : i0 + gl].broadcast_to([P, gl, P]),
                                op=AOT.is_equal,
                            )
                            for jj in range(gl):
                                t = t0 + jj
                                if not (cidx_rng[0] <= t < cidx_rng[1]):
                                    ic0 = (t * 8 // IDXCH) * IDXCH
                                    icl = min(IDXCH, T * 8 - ic0)
                                    cidx = cidxp.tile([P, IDXCH], i16, tag="ci", name="ci")
                                    nc.sync.dma_start(
                                        out=cidx[:, :icl],
                                        in_=idx16_d[:, ic0 : ic0 + icl],
                                    )
                                    cidx_rng = (ic0 // 8, (ic0 + icl) // 8)
                                if not (gch_rng[0] <= t < gch_rng[1]):
                                    # chunks are GCT-aligned within the
                                    # (GCT-padded) sub stream
                                    gc0 = S[s] + ((t - S[s]) // GCT) * GCT
                                    gcl = min(GCT, S[s + 1] - gc0)
                                    ng = gcl * P
                                    gch = cgp.tile(
                                        [P, GCT * F_OUT], bf16, tag="cg", name="cg"
                                    )
                                    ib = (gc0 - cidx_rng[0]) * 8
                                    nc.gpsimd.dma_gather(
                                        out_ap=gch[:, : gcl * F_OUT].rearrange(
                                            "p (n e) -> p n e", e=F_OUT
                                        ),
                                        in_ap=g_subs[s][:, :],
                                        idxs_ap=cidx[:, ib : ib + gcl * 8],
                                        num_idxs=ng,
                                        num_idxs_reg=ng,
                                        elem_size=F_OUT,
                                        single_packet=False,
                                        queue_num=gq[0] % 4,
                                    )
                                    gq[0] += 1
                                    gch_rng = (gc0, gc0 + gcl)
                                gi = t - gch_rng[0]
                                nc.tensor.matmul(
                                    out=pw[:],
                                    lhsT=oh4[:, jj * P : (jj + 1) * P],
                                    rhs=gch[:, gi * F_OUT : (gi + 1) * F_OUT],
                                    start=(j0 + jj == 0),
                                    stop=(j0 + jj == nt - 1),
                                )
                        # accumulate / flush
                        if s == 0:
                            nc.vector.tensor_tensor(
                                out=accw[:, w * F_OUT : (w + 1) * F_OUT],
                                in0=pw[:],
                                in1=gown[:, w * F_OUT : (w + 1) * F_OUT],
                                op=AOT.add,
                            )
                        elif s < NSUB - 1:
                            if nt:
                                nc.vector.tensor_tensor(
                                    out=accw[:, w * F_OUT : (w + 1) * F_OUT],
                                    in0=pw[:],
                                    in1=accw[:, w * F_OUT : (w + 1) * F_OUT],
                                    op=AOT.add,
                                )
                        else:
                            nn = min(P, shard - w * P)
                            acc = cfp.tile([P, F_OUT], f32, tag="facc")
                            if nt:
                                nc.vector.tensor_tensor(
                                    out=acc[:],
                                    in0=pw[:],
                                    in1=accw[:, w * F_OUT : (w + 1) * F_OUT],
                                    op=AOT.add,
                                )
                                accv = acc
                            else:
                                accv = accw[:, w * F_OUT : (w + 1) * F_OUT]
                            accs = cfp.tile([P, F_OUT], f32, tag="faccs")
                            nc.scalar.activation(
                                out=accs[:],
                                in_=accv[:] if accv is acc else accv,
                                func=ACT.Copy,
                                scale=dis_s[:, w : w + 1],
                            )
                            nc.vector.tensor_tensor(
                                out=accs[:], in0=accs[:], in1=biasb[:], op=AOT.add
                            )
                            am = cfp.tile([P, F_OUT], f32, tag="fam")
                            nc.vector.tensor_tensor(
                                out=am[:], in0=accs[:], in1=prelub[:], op=AOT.mult
                            )
                            nc.vector.tensor_tensor(
                                out=am[:], in0=accs[:], in1=am[:], op=AOT.max
                            )
                            nc.sync.dma_start(
                                out=out_d[w * P : w * P + nn, :], in_=am[:nn, :]
                            )

            # interleaved emission: C(s) only needs g-sub s = B shards
            # 2s, 2s+1; B work hides under C's gather/DVE stream.
            phase_b_shard(0)
            phase_b_shard(1)
            phase_own()
            phase_c_sub(0)
            phase_b_shard(2)
            phase_b_shard(3)
            phase_c_sub(1)
            phase_b_shard(4)
            phase_b_shard(5)
            phase_c_sub(2)
            phase_b_shard(6)
            phase_b_shard(7)
            phase_c_sub(3)

    nc.compile()
    return nc


def _get_program(cfg, tsw, debug_outs=False):
    key = (tuple(sorted(cfg.items())), tsw, debug_outs)
    if key not in _prog_cache:
        _prog_cache[key] = build_program(cfg, tsw, debug_outs)
    return _prog_cache[key]


def make_in_maps(prep):
    return [
        {
            "x_t": prep["x_t"],
            "w": prep["w"],
            "bias_b": prep["bias_b"],
            "prelu_b": prep["prelu_b"],
            "deg_lay": prep["deg_lay"],
            "idx16": prep["idx16"][k],
            "colrel_t16": prep["colrel_t16"][k],
        }
        for k in range(CORES)
    ]


def kernel(x, edge_index, W, bias, prelu_a, cfg=None):
    from concourse import bass_utils

    cfg = cfg or FULL_CFG
    prep = host_prep(x, edge_index, W, bias, prelu_a, cfg)
    nc = _get_program(cfg, prep["tsw"])
    res = bass_utils.run_bass_kernel_spmd(
        nc, make_in_maps(prep), core_ids=list(range(CORES))
    )
    out = np.concatenate([res.results[k]["out"] for k in range(CORES)], axis=0)
    return out.astype(np.float32)
